# revision 15
# baseline (speedup 1.0000x reference)
"""Trainium2 Bass kernel for nn_MultiHeadAttention (B=2, S=2048, D=1024, H=16).

Reference semantics (note the *raw-view* head split):
    q = query @ Wq.T + bq                  # [B, S, D]
    q = q.reshape(B, H, S, DK)             # raw view: head h = rows [h*128,(h+1)*128)
    scores = q @ k.T / sqrt(DK), causal mask, softmax
    ctx    = softmax @ v                   # [B, H, S, DK]
    out    = ctx.transpose(0,2,1,3).reshape(B,S,D) @ Wo.T + bo

Sharding: 8 cores = 2 batches x 4 head-groups.  Core (b, g) owns heads
[4g, 4g+4) of batch b = rows [512g, 512g+512) of the QKV projections.  Each
core computes its 4 heads' attention plus its partial out-projection
C_heads @ Wo[:, head cols].T; the host sums 4 partials per batch + bo.

v3 design (vs v2): pair-split q AND k projections (N=256 halves) so
attention starts ~20us in; i-outer matmul emission into multi-bank psum
quads trailing the DMA stream; bias folded into the projection via K=1
matmuls; merged 3D-AP scatters (psum -> q/k pair tiles) split DVE/ACT;
DMA ordered wq,xq0,wk,xk0,xv,wv,xq1,xk1,wo with consts on the ACT ring;
q1/k1/v-proj/out-proj all run as PE fillers inside the attention loop;
ctx matmuls for the first two windows deferred (exp buffered in pt/pt8)
until the v reshape lands.
"""

import os
import sys

import numpy as np

_TRN_REPO = "/opt/trn_rl_repo"
if _TRN_REPO not in sys.path:
    sys.path.insert(0, _TRN_REPO)

B, S, D, H = 2, 2048, 1024, 16
DK = D // H  # 64
N_CORES = 8
HEADS_PER_CORE = 4
ROWS_PER_CORE = 512  # rows of the projection output owned per core
QW = 512  # q-position window (psum free-dim)
KT = 128  # k-position tile


def _build_program(repeat=1, phases=3):
    import concourse.bass as bass
    import concourse.bacc as bacc
    import concourse.mybir as mybir
    from concourse.tile import TileContext
    from contextlib import ExitStack

    f32 = mybir.dt.float32
    bf16 = mybir.dt.bfloat16
    f8 = mybir.dt.float8e4
    DRow = mybir.MatmulPerfMode.DoubleRow
    Exp = mybir.ActivationFunctionType.Exp
    Identity = mybir.ActivationFunctionType.Identity
    MUL = mybir.AluOpType.mult

    nc = bacc.Bacc("TRN2", target_bir_lowering=False, debug=False)

    # ---- DRAM parameters (host pre-tiled / pre-transposed, bf16) ----
    xq = nc.dram_tensor("xq", [8, 128, QW], bf16, kind="ExternalInput")
    xk = nc.dram_tensor("xk", [8, 128, QW], bf16, kind="ExternalInput")
    xv = nc.dram_tensor("xv", [8, 128, QW], bf16, kind="ExternalInput")
    wq = nc.dram_tensor("wq", [8, 2, 128, 512], bf16, kind="ExternalInput")
    wk = nc.dram_tensor("wk", [8, 2, 128, 512], bf16, kind="ExternalInput")
    wv = nc.dram_tensor("wv", [8, 2, 128, 512], bf16, kind="ExternalInput")
    wo = nc.dram_tensor("wo", [2, 128, 1024], bf16, kind="ExternalInput")
    bqr = nc.dram_tensor("bqr", [1, 1024], bf16, kind="ExternalInput")
    bkr = nc.dram_tensor("bkr", [1, 1024], bf16, kind="ExternalInput")
    bvr = nc.dram_tensor("bvr", [1, 1024], bf16, kind="ExternalInput")
    tri = nc.dram_tensor("tri", [128, 256], bf16, kind="ExternalInput")
    ones512 = nc.dram_tensor("ones512", [1, 512], bf16, kind="ExternalInput")
    out = nc.dram_tensor("out", [S, D], bf16, kind="ExternalOutput")
    # per-head DRAM scratch for the v reshape round-trip (vstage layout)
    vscr = [
        nc.dram_tensor(f"vscr{h}", [128, 1024], bf16, kind="Internal")
        for h in range(4)
    ]

    with TileContext(nc) as tc:
      with ExitStack() as stack:
        persist = stack.enter_context(tc.tile_pool(name="persist", bufs=1))
        vhp = stack.enter_context(tc.tile_pool(name="vhp", bufs=1))
        small = stack.enter_context(tc.tile_pool(name="small", bufs=6))
        xp = stack.enter_context(tc.tile_pool(name="xp", bufs=3))
        wp = stack.enter_context(tc.tile_pool(name="wp", bufs=3))
        vsb = stack.enter_context(tc.tile_pool(name="vsb", bufs=1))
        ptp = stack.enter_context(tc.tile_pool(name="ptp", bufs=9))
        ptp8 = stack.enter_context(tc.tile_pool(name="ptp8", bufs=4))
        vh8p = stack.enter_context(tc.tile_pool(name="vh8p", bufs=1))
        wop = stack.enter_context(tc.tile_pool(name="wop", bufs=1))
        osb = stack.enter_context(tc.tile_pool(name="osb", bufs=5))
        for rep in range(repeat):
            # persistent tiles
            qpair = [persist.tile([128, S], bf16, tag=f"qpair{p}", name=f"qpair{p}") for p in range(2)]
            kpair = [persist.tile([128, S], bf16, tag=f"kpair{p}", name=f"kpair{p}") for p in range(2)]
            ctxT = [persist.tile([128, S], bf16, tag=f"ctxT{p}", name=f"ctxT{p}") for p in range(2)]
            tri01 = persist.tile([128, 256], bf16, tag="tri01")
            bq_t = persist.tile([1, 1024], bf16, tag="bq_t")
            bk_t = persist.tile([1, 1024], bf16, tag="bk_t")
            bv_t = persist.tile([1, 1024], bf16, tag="bv_t")
            ones_t = persist.tile([1, 512], bf16, tag="ones_t")

            # vh: one tile per head [128 kpos, 16*(DK+1)]; per ktile j cols
            # [j*65, j*65+64) = v data, col j*65+64 = ones (softmax denom)
            vh = [
                vhp.tile([128, 16 * (DK + 1)], bf16, tag=f"vh_{h}", name=f"vh_{h}")
                for h in range(4)
            ]
            vh8 = [
                vh8p.tile([128, 16 * 128], f8, tag=f"vh8_{h}", name=f"vh8_{h}")
                for h in range(4)
            ]
            vstage = [
                vsb.tile([128, 1024], bf16, tag=f"vst_{rt}", name=f"vst_{rt}")
                for rt in range(4)
            ]

            xq_t = xp.tile([128, 8 * QW], bf16, tag="xall", name="xq_t")
            xk_t = xp.tile([128, 8 * QW], bf16, tag="xall", name="xk_t")
            xv_t = xp.tile([128, 8 * QW], bf16, tag="xall", name="xv_t")
            wq_t = wp.tile([128, 2 * 8 * 512], bf16, tag="wall", name="wq_t")
            wk_t = wp.tile([128, 2 * 8 * 512], bf16, tag="wall", name="wk_t")
            wv_t = wp.tile([128, 2 * 8 * 512], bf16, tag="wall", name="wv_t")
            wo_t = wop.tile([128, 2048], bf16, tag="wo", name="wo_t")

            # ---- DMA issue. SP ring carries the ordered critical stream:
            # wq, xq0, wk, xk0, xv, wv, xq1, xk1, wo.  ACT ring: consts.
            def w_piece(eng, dram_t, sbuf_t, i0, i1):
                # both f-halves for i in [i0, i1); sbuf col layout (i, fh)
                eng.dma_start(
                    out=sbuf_t[:, i0 * 1024 : i1 * 1024].rearrange(
                        "p (x c) -> p x c", c=512
                    ),
                    in_=dram_t[i0:i1].rearrange("i a p c -> p (i a) c"),
                )

            def x_piece(eng, dram_t, sbuf_t, i0, i1, c0, c1):
                eng.dma_start(
                    out=sbuf_t[:].rearrange("p (i c) -> p i c", i=8)[
                        :, i0:i1, c0:c1
                    ],
                    in_=dram_t[i0:i1, :, c0:c1].rearrange("i p c -> p i c"),
                )

            # consts on the SWDGE (Pool) ring so they never steal HWDGE
            # issue slots from the critical SP stream
            nc.gpsimd.dma_start(out=tri01[:], in_=tri[:])
            nc.gpsimd.dma_start(out=bq_t[:], in_=bqr[:])
            nc.gpsimd.dma_start(out=bk_t[:], in_=bkr[:])
            nc.gpsimd.dma_start(out=bv_t[:], in_=bvr[:])
            nc.gpsimd.dma_start(out=ones_t[:], in_=ones512[:])

            # SP ring, interleaved so q0/k0 matmuls trail the stream
            w_piece(nc.sync, wq, wq_t, 0, 1)
            x_piece(nc.sync, xq, xq_t, 0, 2, 0, 256)
            w_piece(nc.sync, wq, wq_t, 1, 3)
            x_piece(nc.sync, xq, xq_t, 2, 5, 0, 256)
            w_piece(nc.sync, wq, wq_t, 3, 6)
            x_piece(nc.sync, xq, xq_t, 5, 8, 0, 256)
            w_piece(nc.sync, wq, wq_t, 6, 8)
            w_piece(nc.sync, wk, wk_t, 0, 2)
            x_piece(nc.sync, xk, xk_t, 0, 4, 0, 256)
            w_piece(nc.sync, wk, wk_t, 2, 5)
            x_piece(nc.sync, xk, xk_t, 4, 8, 0, 256)
            w_piece(nc.sync, wk, wk_t, 5, 8)
            x_piece(nc.sync, xv, xv_t, 0, 8, 0, 512)
            w_piece(nc.sync, wv, wv_t, 0, 4)
            w_piece(nc.sync, wv, wv_t, 4, 8)
            x_piece(nc.sync, xq, xq_t, 0, 8, 256, 512)
            x_piece(nc.sync, xk, xk_t, 0, 8, 256, 512)
            nc.sync.dma_start(
                out=wo_t[:].rearrange("p (a o) -> p a o", a=2),
                in_=wo[:].rearrange("a p o -> p a o"),
            )

            # ones columns of vh (memset once per rep)
            for rt in range(4):
                nc.vector.memset(
                    vh[rt][:].rearrange("p (j e) -> p j e", e=65)[:, :, 64], 1.0
                )

            # preload Exp table early (ACT is idle during phase P)
            dummy = small.tile([1, 4], f32, tag="dummy")
            nc.scalar.activation(dummy[:], tri01[0:1, 0:4], Exp)

            # ---------------- Phase P: pair-0 q/k projections ----------------
            # quad psum [128, 4*w]: col block f4 holds features
            # [fh*512 + f4*128, +128) x s-window [s0, s0+w)
            def emit_proj_quads(quads, w_sb, x_sb, bias_t, w, s0):
                # i-outer across both quads so matmuls trail the DMA stream
                # back-to-back (keeps the PE clock warm)
                # NOTE start=True clears the has_written bits of the WHOLE
                # psum bank, so with two f4 column-blocks per 2KB bank only
                # the bank's first-touching matmul may carry start=True (the
                # other block's first write then lands on cleared bits and
                # overwrites); stop likewise once per bank on its last write.
                for i in range(8):
                    for fh in range(2):
                        for f4 in range(4):
                            nc.tensor.matmul(
                                quads[fh][:, f4 * w : (f4 + 1) * w],
                                w_sb[:, (i * 2 + fh) * 512 + f4 * 128 : (i * 2 + fh) * 512 + (f4 + 1) * 128],
                                x_sb[:, i * 512 + s0 : i * 512 + s0 + w],
                                start=(i == 0 and f4 % 2 == 0),
                                stop=False,
                            )
                # bias via K=1 matmul: psum[f, s] += b[f] * 1
                for fh in range(2):
                    for f4 in range(4):
                        nc.tensor.matmul(
                            quads[fh][:, f4 * w : (f4 + 1) * w],
                            bias_t[0:1, (fh * 4 + f4) * 128 : (fh * 4 + f4 + 1) * 128],
                            ones_t[0:1, 0:w],
                            start=False,
                            stop=(f4 % 2 == 1),
                        )

            def scatter_quad_fh(quad, dest_pair, heads, w, s0, fh, nf=4):
                # merged scatter: per (h, c2) one 3D copy, one ENGINE PER
                # QUAD (fh0 -> DVE, fh1 -> ACT) so both quads scatter
                # concurrently
                # src [64, nf, 128] <- quad[c2-half, (f4, s-block h)]
                # dst [64, nf, 128] -> pair tile cols r*16 + (8fh + 2f4 + c2)
                for h in heads:
                    for c2 in range(2):
                        src = (
                            quad[c2 * 64 : (c2 + 1) * 64, :]
                            .rearrange("p (f s) -> p f s", f=nf)[
                                :, :, h * 128 - s0 : h * 128 - s0 + 128
                            ]
                        )
                        dst = (
                            dest_pair[h // 2][(h % 2) * 64 : (h % 2) * 64 + 64, :]
                            .rearrange("p (r c) -> p c r", c=16)[
                                :, 8 * fh + c2 : 8 * fh + 2 * nf : 2, :
                            ]
                        )
                        if fh == 0:
                            nc.vector.tensor_copy(out=dst, in_=src)
                        else:
                            nc.scalar.activation(dst, src, Identity)

            with tc.tile_pool(name=f"ppsP{rep}", bufs=4, space="PSUM") as ppsP:
                # q pair-0 half: 2 quads [128, 4*256]
                q0 = [ppsP.tile([128, 1024], f32, tag="quad", name=f"q0_{fh}") for fh in range(2)]
                emit_proj_quads(q0, wq_t, xq_t, bq_t, 256, 0)
                for fh in range(2):
                    scatter_quad_fh(q0[fh], qpair, (0, 1), 256, 0, fh)
                k0 = [ppsP.tile([128, 1024], f32, tag="quad", name=f"k0_{fh}") for fh in range(2)]
                emit_proj_quads(k0, wk_t, xk_t, bk_t, 256, 0)
                for fh in range(2):
                    scatter_quad_fh(k0[fh], kpair, (0, 1), 256, 0, fh)

            if phases < 3:
                # debug: stop after phase P, dump q/k pair tiles into out
                for t_i, t in enumerate([qpair[0], kpair[0]]):
                    for half in range(2):
                        nc.sync.dma_start(
                            out=out[(2 * t_i + half) * 128 : (2 * t_i + half + 1) * 128, :],
                            in_=t[:, half * 1024 : (half + 1) * 1024],
                        )
                continue

            # ---------------- Phase A: attention ----------------
            with (
                tc.tile_pool(name=f"scps{rep}", bufs=2, space="PSUM") as scps,
                tc.tile_pool(name=f"ctxps{rep}", bufs=2, space="PSUM") as ctxps,
                tc.tile_pool(name=f"fps{rep}", bufs=2, space="PSUM") as fps,
            ):
                fillers = []   # pair-0-critical: v rt0/rt1, then q1/k1
                fillers2 = []  # pair-1 work: v rt2/rt3, out stiles
                vleft = [0, 0]  # unfinished v fillers per pair

                # ---- filler: second-half (pair-1) q/k projection, in two
                # f4-pair psums per fh, scatters merged over 2 chunks
                def emit_proj1_half(w_sb, x_sb, bias_t, dest_pair, fh, fp):
                    # fp in {0,1}: f4 pair (2*fp, 2*fp+1)
                    ps = fps.tile([128, 512], f32, tag="fill", name="p1")
                    for i in range(8):
                        for f2 in range(2):
                            f4 = 2 * fp + f2
                            nc.tensor.matmul(
                                ps[:, f2 * 256 : (f2 + 1) * 256],
                                w_sb[:, (i * 2 + fh) * 512 + f4 * 128 : (i * 2 + fh) * 512 + (f4 + 1) * 128],
                                x_sb[:, i * 512 + 256 : i * 512 + 512],
                                start=(i == 0 and f2 == 0),
                                stop=False,
                            )
                    for f2 in range(2):
                        f4 = 2 * fp + f2
                        nc.tensor.matmul(
                            ps[:, f2 * 256 : (f2 + 1) * 256],
                            bias_t[0:1, (fh * 4 + f4) * 128 : (fh * 4 + f4 + 1) * 128],
                            ones_t[0:1, 0:256],
                            start=False,
                            stop=(f2 == 1),
                        )
                    flip = fp
                    for h in (2, 3):
                        for c2 in range(2):
                            src = (
                                ps[c2 * 64 : (c2 + 1) * 64, :]
                                .rearrange("p (f s) -> p f s", f=2)[
                                    :, :, h * 128 - 256 : h * 128 - 256 + 128
                                ]
                            )
                            dst = (
                                dest_pair[h // 2][(h % 2) * 64 : (h % 2) * 64 + 64, :]
                                .rearrange("p (r c) -> p c r", c=16)[
                                    :, 8 * fh + 4 * fp + c2 : 8 * fh + 4 * fp + 4 : 2, :
                                ]
                            )
                            if flip % 2 == 0:
                                nc.vector.tensor_copy(out=dst, in_=src)
                            else:
                                nc.scalar.activation(dst, src, Identity)
                            flip += 1

                # ---- filler: v projection sub-chunks ----
                vps_map = {}

                def emit_v_sub(rt, fh, part):
                    # part 0: i 0-3, part 1: i 4-7 + bias + copy to vstage
                    key = (rt, fh)
                    if part == 0:
                        vps_map[key] = fps.tile([128, 512], f32, tag="fill", name="vproj")
                    ps = vps_map[key]
                    for i in range(4 * part, 4 * part + 4):
                        nc.tensor.matmul(
                            ps[:],
                            xv_t[:, i * 512 + rt * 128 : i * 512 + (rt + 1) * 128],
                            wv_t[:, (i * 2 + fh) * 512 : (i * 2 + fh) * 512 + 512],
                            start=(i == 0),
                            stop=False,
                        )
                    if part == 1:
                        nc.tensor.matmul(
                            ps[:],
                            ones_t[0:1, 0:128],
                            bv_t[:, fh * 512 : (fh + 1) * 512],
                            start=False,
                            stop=True,
                        )
                        nc.vector.tensor_copy(
                            out=vstage[rt][:, fh * 512 : (fh + 1) * 512], in_=ps[:]
                        )
                        del vps_map[key]

                def emit_v_reshape(rt):
                    # DRAM round-trip (both hops partition-major = cheap).
                    # Hops ride the two HWDGE rings: lower fixed cost than
                    # SWDGE and the per-rt pipelines overlap across rings.
                    nc.sync.dma_start(out=vscr[rt][:], in_=vstage[rt][:])
                    nc.scalar.dma_start(
                        out=vh[rt][:].rearrange("p (j e) -> p j e", e=65)[:, :, 0:64],
                        in_=vscr[rt][:].rearrange("(j r) (c d) -> r c j d", r=8, d=64),
                    )
                    # fp8 copy (padded 128-col blocks) for DoubleRow ctx
                    nc.vector.tensor_copy(
                        out=vh8[rt][:].rearrange("p (j e) -> p j e", e=128)[:, :, 0:65],
                        in_=vh[rt][:].rearrange("p (j e) -> p j e", e=65),
                    )

                def v_filler(pairidx, mk):
                    def run():
                        mk()
                        vleft[pairidx] -= 1
                    vleft[pairidx] += 1
                    return run

                # filler order: v rt0/rt1 (pair-0 ctx), then q1/k1 halves
                # (pair-1 scores); v rt2/rt3 go to fillers2 (popped in pair 1)
                for rt in range(2):
                    for fh in range(2):
                        for part in range(2):
                            fillers.append(v_filler(0,
                                lambda rt=rt, fh=fh, part=part: emit_v_sub(rt, fh, part)))
                    fillers.append(v_filler(0, lambda rt=rt: emit_v_reshape(rt)))
                for fh in range(2):
                    for fp in range(2):
                        fillers.append(lambda fh=fh, fp=fp: emit_proj1_half(
                            wq_t, xq_t, bq_t, qpair, fh, fp))
                for fh in range(2):
                    for fp in range(2):
                        fillers.append(lambda fh=fh, fp=fp: emit_proj1_half(
                            wk_t, xk_t, bk_t, kpair, fh, fp))
                for rt in range(2, 4):
                    for fh in range(2):
                        for part in range(2):
                            fillers2.append(v_filler(1,
                                lambda rt=rt, fh=fh, part=part: emit_v_sub(rt, fh, part)))
                    fillers2.append(v_filler(1, lambda rt=rt: emit_v_reshape(rt)))

                emitted_st = set()

                def emit_out_stile(st, pool=None, alt_copy=False):
                    emitted_st.add(st)
                    ostage = osb.tile([128, 1024], bf16, tag="ostage", name="ostage")
                    for og in range(2):
                        ps = (pool or fps).tile([128, 512], f32, tag="fill", name="ops")
                        for pair in range(2):
                            nc.tensor.matmul(
                                ps[:],
                                ctxT[pair][:, st * 128 : (st + 1) * 128],
                                wo_t[:, pair * 1024 + og * 512 : pair * 1024 + (og + 1) * 512],
                                start=(pair == 0),
                                stop=(pair == 1),
                            )
                        if alt_copy and og == 1:
                            nc.scalar.activation(
                                ostage[:, og * 512 : (og + 1) * 512], ps[:], Identity
                            )
                        else:
                            nc.vector.tensor_copy(
                                out=ostage[:, og * 512 : (og + 1) * 512], in_=ps[:]
                            )
                    nc.sync.dma_start(
                        out=out[st * 128 : (st + 1) * 128, :], in_=ostage[:]
                    )

                ost_map = {}

                def stile_og(st, og):
                    if og == 0:
                        ost_map[st] = osb.tile(
                            [128, 1024], bf16, tag="ostage", name="ostage"
                        )
                        emitted_st.add(st)
                    ostage = ost_map[st]
                    ps = fps.tile([128, 512], f32, tag="fill", name="ops")
                    for pr in range(2):
                        nc.tensor.matmul(
                            ps[:],
                            ctxT[pr][:, st * 128 : (st + 1) * 128],
                            wo_t[:, pr * 1024 + og * 512 : pr * 1024 + (og + 1) * 512],
                            start=(pr == 0),
                            stop=(pr == 1),
                        )
                    nc.vector.tensor_copy(
                        out=ostage[:, og * 512 : (og + 1) * 512], in_=ps[:]
                    )
                    if og == 1:
                        nc.sync.dma_start(
                            out=out[st * 128 : (st + 1) * 128, :],
                            in_=ostage[:],
                        )
                        del ost_map[st]

                def scores(pair, qi, kj):
                    d = kj - 4 * qi
                    off = max(0, 128 * d)
                    sp = scps.tile([128, 2 * QW], f32, tag="sduo")
                    for h2 in range(2):
                        nc.tensor.matmul(
                            sp[:, h2 * QW + off : (h2 + 1) * QW],
                            kpair[pair][h2 * 64 : h2 * 64 + 64, kj * KT : (kj + 1) * KT],
                            qpair[pair][h2 * 64 : h2 * 64 + 64, qi * QW + off : (qi + 1) * QW],
                            start=True,
                            stop=True,
                        )
                    return sp

                steps = []
                for pair in range(2):
                    for qi in range(4):
                        nkt = 4 * qi + 4
                        for kj in range(nkt):
                            steps.append((pair, qi, kj, nkt))

                # ctx deferral: while pair p's v fillers are pending, hold all
                # of pair p's ctx matmuls (exp output parked in pt/pt8 pools);
                # flush in window order once the reshapes are emitted.  This
                # keeps the in-order PE queue from blocking on the v pipeline.
                deferred = []
                defer_done = [False, False]

                cps_map = {}
                pt8_map = {}
                flushed = [False]

                def get_cps(pair, qi):
                    if (pair, qi) not in cps_map:
                        cps_map[(pair, qi)] = [
                            ctxps.tile([DK + 1, QW], f32, tag=f"ctx{h2}",
                                       name=f"ctx{h2}", bufs=1)
                            for h2 in range(2)
                        ]
                    return cps_map[(pair, qi)]

                def normalize(pair, qi):
                    cps = cps_map[(pair, qi)]
                    for h2 in range(2):
                        rec = small.tile([1, QW], f32, tag="rec")
                        nc.vector.reciprocal(rec[:], cps[h2][64:65, :])
                        bc = small.tile([64, QW], f32, tag="bc")
                        nc.gpsimd.partition_broadcast(bc[:], rec[:], channels=64)
                        nc.vector.tensor_tensor(
                            out=ctxT[pair][
                                h2 * 64 : h2 * 64 + 64, qi * QW : (qi + 1) * QW
                            ],
                            in0=cps[h2][0:64, :],
                            in1=bc[:],
                            op=MUL,
                        )
                    del cps_map[(pair, qi)]

                def emit_scores(idx):
                    if idx >= len(steps):
                        return None
                    if steps[idx][0] == 1 and not flushed[0]:
                        # pair-1 scores need qpair[1]/kpair[1]: drain the
                        # pair-0-critical filler list first
                        while fillers:
                            fillers.pop(0)()
                        flushed[0] = True
                    return scores(*steps[idx][:3])

                squeue = [emit_scores(0)]
                for si, (pair, qi, kj, nkt) in enumerate(steps):
                    s_cur = squeue.pop(0)
                    squeue.append(emit_scores(si + 1))
                    want_defer = not defer_done[pair]
                    d = kj - 4 * qi
                    off = max(0, 128 * d)
                    if d < 0:
                        # off-diagonal: exp into one half of a paired fp8
                        # tile; the pair fuses into ONE DoubleRow ctx matmul
                        # at the odd step (accuracy-safe off the diagonal)
                        e = kj & 1
                        if e == 0:
                            pt8_map[(pair, qi)] = ptp8.tile(
                                [128, 2048], f8, tag="pt8", name="pt8"
                            )
                        pt8 = pt8_map[(pair, qi)]
                        nc.scalar.activation(
                            pt8[:, e * 1024 : (e + 1) * 1024], s_cur[:], Exp
                        )
                        if e == 1:
                            def drow_ctx(pair=pair, qi=qi, kj=kj, pt8=pt8):
                                cps = get_cps(pair, qi)
                                r3 = pt8[:].rearrange("p (t h x) -> p h t x", t=2, h=2)
                                for h2 in range(2):
                                    h = 2 * pair + h2
                                    nc.tensor.matmul(
                                        cps[h2][:],
                                        vh8[h][:, (kj - 1) * 128 : (kj + 1) * 128]
                                        .rearrange("p (t m) -> p t m", t=2)[:, :, 0:65],
                                        r3[:, h2],
                                        start=(kj == 1),
                                        stop=False,
                                        perf_mode=DRow,
                                    )
                            if want_defer:
                                deferred.append(drow_ctx)
                            else:
                                drow_ctx()
                    else:
                        pt = ptp.tile([128, 2 * QW], bf16, tag="ptduo")
                        s3 = s_cur[:].rearrange("p (h x) -> p h x", h=2)[:, :, off:]
                        p3 = pt[:].rearrange("p (h x) -> p h x", h=2)[:, :, off:]
                        nc.scalar.activation(p3, s3, Exp)
                        # both heads' diagonal masks in one DVE instr
                        # (tri01 holds two side-by-side copies of the mask)
                        mv = pt[:].rearrange("p (h x) -> p h x", h=2)[
                            :, :, off : off + 128
                        ]
                        nc.vector.tensor_tensor(
                            out=mv,
                            in0=mv,
                            in1=tri01[:].rearrange("p (h x) -> p h x", h=2),
                            op=MUL,
                        )

                        def diag_ctx(pair=pair, qi=qi, kj=kj, nkt=nkt, off=off, pt=pt):
                            cps = get_cps(pair, qi)
                            for h2 in range(2):
                                h = 2 * pair + h2
                                nc.tensor.matmul(
                                    cps[h2][:, off:] if off else cps[h2][:],
                                    vh[h][:, kj * 65 : kj * 65 + 65],
                                    pt[:, h2 * QW + off : (h2 + 1) * QW],
                                    start=(kj == 0),
                                    stop=(kj == nkt - 1),
                                )
                        if want_defer:
                            deferred.append(diag_ctx)
                        else:
                            diag_ctx()
                    if kj == nkt - 1:
                        def win_end(pair=pair, qi=qi):
                            normalize(pair, qi)
                            if pair == 1 and qi < 3:
                                # out stiles for this window become fillers;
                                # emitted here (after normalize) so the read
                                # of ctxT is sequenced after its write
                                for st in range(qi * 4, qi * 4 + 4):
                                    fillers2.append(lambda st=st: stile_og(st, 0))
                                    fillers2.append(lambda st=st: stile_og(st, 1))
                        if want_defer:
                            deferred.append(win_end)
                        else:
                            win_end()
                    # pop fillers at step END: their DVE copies land after
                    # this step's mask/normalize in the in-order DVE queue,
                    # keeping the exp->mask->ctx critical path unobstructed
                    pops = 2 if vleft[pair] > 0 else (1 if si % 2 == 1 else 0)
                    for _ in range(pops):
                        if fillers:
                            fillers.pop(0)()
                        elif pair == 1 and fillers2:
                            fillers2.pop(0)()
                    # flush deferred ctx (window order) once pair's v is in
                    if not defer_done[pair] and vleft[pair] == 0:
                        for th in deferred:
                            th()
                        deferred.clear()
                        defer_done[pair] = True
                while fillers:
                    fillers.pop(0)()
                for th in deferred:
                    th()
                deferred.clear()
                while fillers2:
                    fillers2.pop(0)()

            # tail: remaining out stiles in a fresh triple-buffered psum
            # scope (attention pools closed -> banks free), copies split
            # DVE/ACT so the stile chain pipelines
            with tc.tile_pool(name=f"tps{rep}", bufs=3, space="PSUM") as tps:
                for st in range(16):
                    if st not in emitted_st:
                        emit_out_stile(st, pool=tps, alt_copy=True)

    nc.finalize()
    return nc


_NC_CACHE = {}


def _get_program(repeat=1):
    phases = int(os.environ.get("KERNEL_PHASES", "3"))
    key = (repeat, phases)
    if key not in _NC_CACHE:
        _NC_CACHE[key] = _build_program(repeat, phases)
    return _NC_CACHE[key]


def _host_inputs(query, key, value, Wq, bq, Wk, bk, Wv, bv, Wo):
    """Build the 8 per-core input maps (numpy, host-side shard/transpose)."""
    import ml_dtypes

    bf16 = ml_dtypes.bfloat16
    query = np.asarray(query, dtype=np.float32)
    key = np.asarray(key, dtype=np.float32)
    value = np.asarray(value, dtype=np.float32)
    Wq = np.asarray(Wq, dtype=np.float32)
    Wk = np.asarray(Wk, dtype=np.float32)
    Wv = np.asarray(Wv, dtype=np.float32)
    Wo = np.asarray(Wo, dtype=np.float32)
    bq = np.asarray(bq, dtype=np.float32)
    bk = np.asarray(bk, dtype=np.float32)
    bv = np.asarray(bv, dtype=np.float32)

    scale = 1.0 / np.sqrt(np.float32(DK))

    def wtile(WT):  # [1024 i, 1024 f] -> [8, 2, 128, 512] (i-tile, f-half)
        return np.ascontiguousarray(
            WT.reshape(8, 128, 2, 512).transpose(0, 2, 1, 3)
        ).astype(bf16)

    wq4 = wtile(Wq.T * scale)
    wk4 = wtile(Wk.T)
    wv4 = wtile(Wv.T)
    WoT = np.ascontiguousarray(Wo.T)  # [i, o]

    bqr = (bq * scale).reshape(1, 1024).astype(bf16)
    bkr = bk.reshape(1, 1024).astype(bf16)
    bvr = bv.reshape(1, 1024).astype(bf16)
    t1 = np.triu(np.ones((128, 128), np.float32))
    tri01 = np.ascontiguousarray(np.concatenate([t1, t1], axis=1)).astype(bf16)

    in_maps = []
    for core in range(N_CORES):
        b, g = divmod(core, 4)
        sl = slice(g * ROWS_PER_CORE, (g + 1) * ROWS_PER_CORE)
        xq_ = np.ascontiguousarray(query[b, sl, :].T).astype(bf16).reshape(8, 128, QW)
        xk_ = np.ascontiguousarray(key[b, sl, :].T).astype(bf16).reshape(8, 128, QW)
        xv_ = np.ascontiguousarray(value[b, sl, :].T).astype(bf16).reshape(8, 128, QW)
        wo4 = np.ascontiguousarray(WoT[g * 256 : (g + 1) * 256, :]).astype(bf16).reshape(2, 128, 1024)
        in_maps.append(
            {
                "ones512": np.ones((1, 512), bf16),
                "xq": xq_,
                "xk": xk_,
                "xv": xv_,
                "wq": wq4,
                "wk": wk4,
                "wv": wv4,
                "wo": wo4,
                "bqr": bqr,
                "bkr": bkr,
                "bvr": bvr,
                "tri": tri01,
            }
        )
    return in_maps


def run_cores(in_maps, trace=False, trace_kwargs=None, repeat=1):
    """Compile + run the SPMD program on cores 0-7, return BassKernelResults."""
    from concourse.bass_utils import run_bass_kernel_spmd

    nc = _get_program(repeat)
    kwargs = {}
    if trace:
        kwargs["trace"] = True
        if trace_kwargs:
            kwargs["trace_kwargs"] = trace_kwargs
    return run_bass_kernel_spmd(nc, in_maps, core_ids=list(range(N_CORES)), **kwargs)


def kernel(query, key, value, mask, Wq, bq, Wk, bk, Wv, bv, Wo, bo, _trace=False):
    in_maps = _host_inputs(query, key, value, Wq, bq, Wk, bk, Wv, bv, Wo)
    res = run_cores(in_maps, trace=_trace)
    bo = np.asarray(bo, dtype=np.float32)
    out = np.zeros((B, S, D), dtype=np.float32)
    for core in range(N_CORES):
        b = core // 4
        out[b] += np.asarray(res.results[core]["out"], dtype=np.float32)
    out += bo[None, None, :]
    kernel.last_results = res
    return out


# revision 27
# speedup vs baseline: 1.2781x; 1.2781x over previous
"""Trainium2 Bass kernel for nn_MultiHeadAttention (B=2, S=2048, D=1024, H=16).

Reference semantics (note the *raw-view* head split):
    q = query @ Wq.T + bq                  # [B, S, D]
    q = q.reshape(B, H, S, DK)             # raw view: head h = rows [h*128,(h+1)*128)
    scores = q @ k.T / sqrt(DK), causal mask, softmax
    ctx    = softmax @ v                   # [B, H, S, DK]
    out    = ctx.transpose(0,2,1,3).reshape(B,S,D) @ Wo.T + bo

Sharding: 8 cores = 2 batches x 4 head-groups.  Core (b, g) owns heads
[4g, 4g+4) of batch b = rows [512g, 512g+512) of the QKV projections.  Each
core computes its 4 heads' attention plus its partial out-projection
C_heads @ Wo[:, head cols].T; the host sums 4 partials per batch + bo.

v3 design (vs v2): pair-split q AND k projections (N=256 halves) so
attention starts ~20us in; i-outer matmul emission into multi-bank psum
quads trailing the DMA stream; bias folded into the projection via K=1
matmuls; merged 3D-AP scatters (psum -> q/k pair tiles) split DVE/ACT;
DMA ordered wq,xq0,wk,xk0,xv,wv,xq1,xk1,wo with consts on the ACT ring;
q1/k1/v-proj/out-proj all run as PE fillers inside the attention loop;
ctx matmuls for the first two windows deferred (exp buffered in pt/pt8)
until the v reshape lands.
"""

import os
import sys

import numpy as np

_TRN_REPO = "/opt/trn_rl_repo"
if _TRN_REPO not in sys.path:
    sys.path.insert(0, _TRN_REPO)

B, S, D, H = 2, 2048, 1024, 16
DK = D // H  # 64
N_CORES = 8
HEADS_PER_CORE = 4
ROWS_PER_CORE = 512  # rows of the projection output owned per core
QW = 512  # q-position window (psum free-dim)
KT = 128  # k-position tile


def _build_program(repeat=1, phases=3):
    import concourse.bass as bass
    import concourse.bacc as bacc
    import concourse.mybir as mybir
    from concourse.tile import TileContext
    from contextlib import ExitStack

    f32 = mybir.dt.float32
    bf16 = mybir.dt.bfloat16
    f8 = mybir.dt.float8e4
    DRow = mybir.MatmulPerfMode.DoubleRow
    Exp = mybir.ActivationFunctionType.Exp
    Identity = mybir.ActivationFunctionType.Identity
    MUL = mybir.AluOpType.mult

    nc = bacc.Bacc("TRN2", target_bir_lowering=False, debug=False)

    # ---- DRAM parameters (host pre-tiled / pre-transposed, bf16) ----
    xq = nc.dram_tensor("xq", [8, 128, QW], bf16, kind="ExternalInput")
    xk = nc.dram_tensor("xk", [8, 128, QW], bf16, kind="ExternalInput")
    xv = nc.dram_tensor("xv", [8, 128, QW], bf16, kind="ExternalInput")
    wq = nc.dram_tensor("wq", [8, 2, 128, 512], bf16, kind="ExternalInput")
    wk = nc.dram_tensor("wk", [8, 2, 128, 512], bf16, kind="ExternalInput")
    wv = nc.dram_tensor("wv", [8, 2, 128, 512], bf16, kind="ExternalInput")
    wo = nc.dram_tensor("wo", [2, 128, 1024], bf16, kind="ExternalInput")
    bqr = nc.dram_tensor("bqr", [1, 1024], bf16, kind="ExternalInput")
    bkr = nc.dram_tensor("bkr", [1, 1024], bf16, kind="ExternalInput")
    bvr = nc.dram_tensor("bvr", [1, 1024], bf16, kind="ExternalInput")
    tri = nc.dram_tensor("tri", [128, 256], bf16, kind="ExternalInput")
    ones512 = nc.dram_tensor("ones512", [1, 512], bf16, kind="ExternalInput")
    out = nc.dram_tensor("out", [S, D], bf16, kind="ExternalOutput")
    # per-head DRAM scratch for the v reshape round-trip (vstage layout)
    vscr = [
        nc.dram_tensor(f"vscr{h}", [128, 1024], bf16, kind="Internal")
        for h in range(4)
    ]

    with TileContext(nc) as tc:
      with ExitStack() as stack:
        persist = stack.enter_context(tc.tile_pool(name="persist", bufs=1))
        vhp = stack.enter_context(tc.tile_pool(name="vhp", bufs=1))
        small = stack.enter_context(tc.tile_pool(name="small", bufs=6))
        xp = stack.enter_context(tc.tile_pool(name="xp", bufs=3))
        wp = stack.enter_context(tc.tile_pool(name="wp", bufs=3))
        vsb = stack.enter_context(tc.tile_pool(name="vsb", bufs=1))
        ptp = stack.enter_context(tc.tile_pool(name="ptp", bufs=9))
        ptp8 = stack.enter_context(tc.tile_pool(name="ptp8", bufs=4))
        vh8p = stack.enter_context(tc.tile_pool(name="vh8p", bufs=1))
        wop = stack.enter_context(tc.tile_pool(name="wop", bufs=1))
        osb = stack.enter_context(tc.tile_pool(name="osb", bufs=5))
        for rep in range(repeat):
            # persistent tiles
            qpair = [persist.tile([128, S], bf16, tag=f"qpair{p}", name=f"qpair{p}") for p in range(2)]
            kpair = [persist.tile([128, S], bf16, tag=f"kpair{p}", name=f"kpair{p}") for p in range(2)]
            ctxT = [persist.tile([128, S], bf16, tag=f"ctxT{p}", name=f"ctxT{p}") for p in range(2)]
            tri01 = persist.tile([128, 256], bf16, tag="tri01")
            bq_t = persist.tile([1, 1024], bf16, tag="bq_t")
            bk_t = persist.tile([1, 1024], bf16, tag="bk_t")
            bv_t = persist.tile([1, 1024], bf16, tag="bv_t")
            ones_t = persist.tile([1, 512], bf16, tag="ones_t")

            # vh: one tile per head [128 kpos, 16*(DK+1)]; per ktile j cols
            # [j*65, j*65+64) = v data, col j*65+64 = ones (softmax denom)
            vh = [
                vhp.tile([128, 16 * (DK + 1)], bf16, tag=f"vh_{h}", name=f"vh_{h}")
                for h in range(4)
            ]
            vh8 = [
                vh8p.tile([128, 16 * 128], f8, tag=f"vh8_{h}", name=f"vh8_{h}")
                for h in range(4)
            ]
            vstage = [
                vsb.tile([128, 1024], bf16, tag=f"vst_{rt}", name=f"vst_{rt}")
                for rt in range(4)
            ]

            xq_t = xp.tile([128, 8 * QW], bf16, tag="xall", name="xq_t")
            xk_t = xp.tile([128, 8 * QW], bf16, tag="xall", name="xk_t")
            xv_t = xp.tile([128, 8 * QW], bf16, tag="xall", name="xv_t")
            wq_t = wp.tile([128, 2 * 8 * 512], bf16, tag="wall", name="wq_t")
            wk_t = wp.tile([128, 2 * 8 * 512], bf16, tag="wall", name="wk_t")
            wv_t = wp.tile([128, 2 * 8 * 512], bf16, tag="wall", name="wv_t")
            wo_t = wop.tile([128, 2048], bf16, tag="wo", name="wo_t")

            # ---- DMA issue. SP ring carries the ordered critical stream:
            # wq, xq0, wk, xk0, xv, wv, xq1, xk1, wo.  ACT ring: consts.
            def w_piece(eng, dram_t, sbuf_t, i0, i1):
                # both f-halves for i in [i0, i1); sbuf col layout (i, fh)
                eng.dma_start(
                    out=sbuf_t[:, i0 * 1024 : i1 * 1024].rearrange(
                        "p (x c) -> p x c", c=512
                    ),
                    in_=dram_t[i0:i1].rearrange("i a p c -> p (i a) c"),
                )

            def x_piece(eng, dram_t, sbuf_t, i0, i1, c0, c1):
                eng.dma_start(
                    out=sbuf_t[:].rearrange("p (i c) -> p i c", i=8)[
                        :, i0:i1, c0:c1
                    ],
                    in_=dram_t[i0:i1, :, c0:c1].rearrange("i p c -> p i c"),
                )

            # consts on the SWDGE (Pool) ring so they never steal HWDGE
            # issue slots from the critical SP stream
            nc.gpsimd.dma_start(out=tri01[:], in_=tri[:])
            nc.gpsimd.dma_start(out=bq_t[:], in_=bqr[:])
            nc.gpsimd.dma_start(out=bk_t[:], in_=bkr[:])
            nc.gpsimd.dma_start(out=bv_t[:], in_=bvr[:])
            nc.gpsimd.dma_start(out=ones_t[:], in_=ones512[:])

            # SP ring, interleaved so q0/k0 matmuls trail the stream
            w_piece(nc.sync, wq, wq_t, 0, 1)
            x_piece(nc.sync, xq, xq_t, 0, 2, 0, 256)
            w_piece(nc.sync, wq, wq_t, 1, 3)
            x_piece(nc.sync, xq, xq_t, 2, 5, 0, 256)
            w_piece(nc.sync, wq, wq_t, 3, 6)
            x_piece(nc.sync, xq, xq_t, 5, 8, 0, 256)
            w_piece(nc.sync, wq, wq_t, 6, 8)
            w_piece(nc.sync, wk, wk_t, 0, 2)
            x_piece(nc.sync, xk, xk_t, 0, 4, 0, 256)
            w_piece(nc.sync, wk, wk_t, 2, 5)
            x_piece(nc.sync, xk, xk_t, 4, 8, 0, 256)
            w_piece(nc.sync, wk, wk_t, 5, 8)
            x_piece(nc.sync, xv, xv_t, 0, 8, 0, 512)
            w_piece(nc.sync, wv, wv_t, 0, 4)
            w_piece(nc.sync, wv, wv_t, 4, 8)
            x_piece(nc.sync, xq, xq_t, 0, 8, 256, 512)
            x_piece(nc.sync, xk, xk_t, 0, 8, 256, 512)
            nc.sync.dma_start(
                out=wo_t[:].rearrange("p (a o) -> p a o", a=2),
                in_=wo[:].rearrange("a p o -> p a o"),
            )

            # ones columns of vh (memset once per rep)
            for rt in range(4):
                nc.vector.memset(
                    vh[rt][:].rearrange("p (j e) -> p j e", e=65)[:, :, 64], 1.0
                )

            # preload Exp table early (ACT is idle during phase P)
            dummy = small.tile([1, 4], f32, tag="dummy")
            nc.scalar.activation(dummy[:], tri01[0:1, 0:4], Exp)

            # ---------------- Phase P: pair-0 q/k projections ----------------
            # quad psum [128, 4*w]: col block f4 holds features
            # [fh*512 + f4*128, +128) x s-window [s0, s0+w)
            def emit_proj_quads(quads, w_sb, x_sb, bias_t, w, s0):
                # i-outer across both quads so matmuls trail the DMA stream
                # back-to-back (keeps the PE clock warm)
                # NOTE start=True clears the has_written bits of the WHOLE
                # psum bank, so with two f4 column-blocks per 2KB bank only
                # the bank's first-touching matmul may carry start=True (the
                # other block's first write then lands on cleared bits and
                # overwrites); stop likewise once per bank on its last write.
                for i in range(8):
                    for fh in range(2):
                        for f4 in range(4):
                            nc.tensor.matmul(
                                quads[fh][:, f4 * w : (f4 + 1) * w],
                                w_sb[:, (i * 2 + fh) * 512 + f4 * 128 : (i * 2 + fh) * 512 + (f4 + 1) * 128],
                                x_sb[:, i * 512 + s0 : i * 512 + s0 + w],
                                start=(i == 0 and f4 % 2 == 0),
                                stop=False,
                            )
                # bias via K=1 matmul: psum[f, s] += b[f] * 1
                for fh in range(2):
                    for f4 in range(4):
                        nc.tensor.matmul(
                            quads[fh][:, f4 * w : (f4 + 1) * w],
                            bias_t[0:1, (fh * 4 + f4) * 128 : (fh * 4 + f4 + 1) * 128],
                            ones_t[0:1, 0:w],
                            start=False,
                            stop=(f4 % 2 == 1),
                        )

            def scatter_quads(quads, dest_pair, heads, w, s0, nf=4):
                # merged scatter: per (h, fh, c2) one 3D copy, emitted
                # HEAD-major with alternating engines so the first head's
                # columns complete first (subtile deps unblock the first
                # scores matmul early)
                # src [64, nf, 128] <- quad[c2-half, (f4, s-block h)]
                # dst [64, nf, 128] -> pair tile cols r*16 + (8fh + 2f4 + c2)
                for h in heads:
                    for fh in range(2):
                        for c2 in range(2):
                            src = (
                                quads[fh][c2 * 64 : (c2 + 1) * 64, :]
                                .rearrange("p (f s) -> p f s", f=nf)[
                                    :, :, h * 128 - s0 : h * 128 - s0 + 128
                                ]
                            )
                            dst = (
                                dest_pair[h // 2][(h % 2) * 64 : (h % 2) * 64 + 64, :]
                                .rearrange("p (r c) -> p c r", c=16)[
                                    :, 8 * fh + c2 : 8 * fh + 2 * nf : 2, :
                                ]
                            )
                            if fh == 0:
                                nc.vector.tensor_copy(out=dst, in_=src)
                            else:
                                nc.scalar.activation(dst, src, Identity)

            with tc.tile_pool(name=f"ppsP{rep}", bufs=4, space="PSUM") as ppsP:
                # q pair-0 half: 2 quads [128, 4*256]
                q0 = [ppsP.tile([128, 1024], f32, tag="quad", name=f"q0_{fh}") for fh in range(2)]
                emit_proj_quads(q0, wq_t, xq_t, bq_t, 256, 0)
                scatter_quads(q0, qpair, (0, 1), 256, 0)
                k0 = [ppsP.tile([128, 1024], f32, tag="quad", name=f"k0_{fh}") for fh in range(2)]
                emit_proj_quads(k0, wk_t, xk_t, bk_t, 256, 0)
                scatter_quads(k0, kpair, (0, 1), 256, 0)

            if phases < 3:
                # debug: stop after phase P, dump q/k pair tiles into out
                for t_i, t in enumerate([qpair[0], kpair[0]]):
                    for half in range(2):
                        nc.sync.dma_start(
                            out=out[(2 * t_i + half) * 128 : (2 * t_i + half + 1) * 128, :],
                            in_=t[:, half * 1024 : (half + 1) * 1024],
                        )
                continue

            # ---------------- Phase A: attention ----------------
            with (
                tc.tile_pool(name=f"scps{rep}", bufs=2, space="PSUM") as scps,
                tc.tile_pool(name=f"ctxps{rep}", bufs=2, space="PSUM") as ctxps,
                tc.tile_pool(name=f"fps{rep}", bufs=2, space="PSUM") as fps,
            ):
                fillers = []   # pair-0-critical: v rt0/rt1, then q1/k1
                fillers2 = []  # pair-1 work: v rt2/rt3, out stiles
                vleft = [0, 0]  # unfinished v fillers per pair

                # ---- filler: second-half (pair-1) q/k projection, in two
                # f4-pair psums per fh, scatters merged over 2 chunks
                def emit_proj1_half(w_sb, x_sb, bias_t, dest_pair, fh, fp):
                    # fp in {0,1}: f4 pair (2*fp, 2*fp+1)
                    ps = fps.tile([128, 512], f32, tag="fill", name="p1")
                    for i in range(8):
                        for f2 in range(2):
                            f4 = 2 * fp + f2
                            nc.tensor.matmul(
                                ps[:, f2 * 256 : (f2 + 1) * 256],
                                w_sb[:, (i * 2 + fh) * 512 + f4 * 128 : (i * 2 + fh) * 512 + (f4 + 1) * 128],
                                x_sb[:, i * 512 + 256 : i * 512 + 512],
                                start=(i == 0 and f2 == 0),
                                stop=False,
                            )
                    for f2 in range(2):
                        f4 = 2 * fp + f2
                        nc.tensor.matmul(
                            ps[:, f2 * 256 : (f2 + 1) * 256],
                            bias_t[0:1, (fh * 4 + f4) * 128 : (fh * 4 + f4 + 1) * 128],
                            ones_t[0:1, 0:256],
                            start=False,
                            stop=(f2 == 1),
                        )
                    flip = fp
                    for h in (2, 3):
                        for c2 in range(2):
                            src = (
                                ps[c2 * 64 : (c2 + 1) * 64, :]
                                .rearrange("p (f s) -> p f s", f=2)[
                                    :, :, h * 128 - 256 : h * 128 - 256 + 128
                                ]
                            )
                            dst = (
                                dest_pair[h // 2][(h % 2) * 64 : (h % 2) * 64 + 64, :]
                                .rearrange("p (r c) -> p c r", c=16)[
                                    :, 8 * fh + 4 * fp + c2 : 8 * fh + 4 * fp + 4 : 2, :
                                ]
                            )
                            if flip % 2 == 0:
                                nc.vector.tensor_copy(out=dst, in_=src)
                            else:
                                nc.scalar.activation(dst, src, Identity)
                            flip += 1

                # ---- filler: v projection sub-chunks ----
                vps_map = {}

                def emit_v_sub(rt, fh, part):
                    # part 0: i 0-3, part 1: i 4-7 + bias + copy to vstage
                    key = (rt, fh)
                    if part == 0:
                        vps_map[key] = fps.tile([128, 512], f32, tag="fill", name="vproj")
                    ps = vps_map[key]
                    for i in range(4 * part, 4 * part + 4):
                        nc.tensor.matmul(
                            ps[:],
                            xv_t[:, i * 512 + rt * 128 : i * 512 + (rt + 1) * 128],
                            wv_t[:, (i * 2 + fh) * 512 : (i * 2 + fh) * 512 + 512],
                            start=(i == 0),
                            stop=False,
                        )
                    if part == 1:
                        nc.tensor.matmul(
                            ps[:],
                            ones_t[0:1, 0:128],
                            bv_t[:, fh * 512 : (fh + 1) * 512],
                            start=False,
                            stop=True,
                        )
                        nc.vector.tensor_copy(
                            out=vstage[rt][:, fh * 512 : (fh + 1) * 512], in_=ps[:]
                        )
                        del vps_map[key]

                def emit_v_reshape(rt):
                    # DRAM round-trip (both hops partition-major = cheap).
                    # Hops ride the two HWDGE rings: lower fixed cost than
                    # SWDGE and the per-rt pipelines overlap across rings.
                    nc.sync.dma_start(out=vscr[rt][:], in_=vstage[rt][:])
                    nc.scalar.dma_start(
                        out=vh[rt][:].rearrange("p (j e) -> p j e", e=65)[:, :, 0:64],
                        in_=vscr[rt][:].rearrange("(j r) (c d) -> r c j d", r=8, d=64),
                    )
                    # fp8 copy (padded 128-col blocks) for DoubleRow ctx
                    nc.vector.tensor_copy(
                        out=vh8[rt][:].rearrange("p (j e) -> p j e", e=128)[:, :, 0:65],
                        in_=vh[rt][:].rearrange("p (j e) -> p j e", e=65),
                    )

                def v_filler(pairidx, mk):
                    def run():
                        mk()
                        vleft[pairidx] -= 1
                    vleft[pairidx] += 1
                    return run

                # filler order: v rt0/rt1 (pair-0 ctx), then q1/k1 halves
                # (pair-1 scores); v rt2/rt3 go to fillers2 (popped in pair 1)
                for rt in range(2):
                    for fh in range(2):
                        for part in range(2):
                            fillers.append(v_filler(0,
                                lambda rt=rt, fh=fh, part=part: emit_v_sub(rt, fh, part)))
                    fillers.append(v_filler(0, lambda rt=rt: emit_v_reshape(rt)))
                for fh in range(2):
                    for fp in range(2):
                        fillers.append(lambda fh=fh, fp=fp: emit_proj1_half(
                            wq_t, xq_t, bq_t, qpair, fh, fp))
                for fh in range(2):
                    for fp in range(2):
                        fillers.append(lambda fh=fh, fp=fp: emit_proj1_half(
                            wk_t, xk_t, bk_t, kpair, fh, fp))
                for rt in range(2, 4):
                    for fh in range(2):
                        for part in range(2):
                            fillers2.append(v_filler(1,
                                lambda rt=rt, fh=fh, part=part: emit_v_sub(rt, fh, part)))
                    fillers2.append(v_filler(1, lambda rt=rt: emit_v_reshape(rt)))

                emitted_st = set()

                def emit_out_stile(st, pool=None, alt_copy=False):
                    emitted_st.add(st)
                    ostage = osb.tile([128, 1024], bf16, tag="ostage", name="ostage")
                    for og in range(2):
                        ps = (pool or fps).tile([128, 512], f32, tag="fill", name="ops")
                        for pair in range(2):
                            nc.tensor.matmul(
                                ps[:],
                                ctxT[pair][:, st * 128 : (st + 1) * 128],
                                wo_t[:, pair * 1024 + og * 512 : pair * 1024 + (og + 1) * 512],
                                start=(pair == 0),
                                stop=(pair == 1),
                            )
                        if alt_copy and og == 1:
                            nc.scalar.activation(
                                ostage[:, og * 512 : (og + 1) * 512], ps[:], Identity
                            )
                        else:
                            nc.vector.tensor_copy(
                                out=ostage[:, og * 512 : (og + 1) * 512], in_=ps[:]
                            )
                    nc.sync.dma_start(
                        out=out[st * 128 : (st + 1) * 128, :], in_=ostage[:]
                    )

                ost_map = {}

                def stile_og(st, og, split_dma=False):
                    if og == 0:
                        ost_map[st] = osb.tile(
                            [128, 1024], bf16, tag="ostage", name="ostage"
                        )
                        emitted_st.add(st)
                    ostage = ost_map[st]
                    ps = fps.tile([128, 512], f32, tag="fill", name="ops")
                    for pr in range(2):
                        nc.tensor.matmul(
                            ps[:],
                            ctxT[pr][:, st * 128 : (st + 1) * 128],
                            wo_t[:, pr * 1024 + og * 512 : pr * 1024 + (og + 1) * 512],
                            start=(pr == 0),
                            stop=(pr == 1),
                        )
                    nc.vector.tensor_copy(
                        out=ostage[:, og * 512 : (og + 1) * 512], in_=ps[:]
                    )
                    if split_dma:
                        nc.sync.dma_start(
                            out=out[st * 128 : (st + 1) * 128, og * 512 : (og + 1) * 512],
                            in_=ostage[:, og * 512 : (og + 1) * 512],
                        )
                        if og == 1:
                            del ost_map[st]
                    elif og == 1:
                        nc.sync.dma_start(
                            out=out[st * 128 : (st + 1) * 128, :],
                            in_=ostage[:],
                        )
                        del ost_map[st]

                def scores(pair, qi, kj, packed=False):
                    sp = scps.tile([128, 2 * QW], f32, tag="sduo")
                    if not packed:
                        d = kj - 4 * qi
                        off = max(0, 128 * d)
                        for h2 in range(2):
                            nc.tensor.matmul(
                                sp[:, h2 * QW + off : (h2 + 1) * QW],
                                kpair[pair][h2 * 64 : h2 * 64 + 64, kj * KT : (kj + 1) * KT],
                                qpair[pair][h2 * 64 : h2 * 64 + 64, qi * QW + off : (qi + 1) * QW],
                                start=True,
                                stop=True,
                            )
                        return sp
                    # packed: kj = 4qi+2 (256 wide) and kj+1 (128 wide) share
                    # the duo: sub-step b at cols h2*512 + [256*b, 256*b+256-128*b)
                    for h2 in range(2):
                        for b in range(2):
                            kjj = kj + b
                            off = 128 * (kjj - 4 * qi)
                            w = QW - off
                            nc.tensor.matmul(
                                sp[:, h2 * QW + 256 * b : h2 * QW + 256 * b + w],
                                kpair[pair][h2 * 64 : h2 * 64 + 64, kjj * KT : (kjj + 1) * KT],
                                qpair[pair][h2 * 64 : h2 * 64 + 64, qi * QW + off : (qi + 1) * QW],
                                start=(b == 0),
                                stop=(b == 1),
                            )
                    return sp

                steps = []
                for pair in range(2):
                    for qi in range(4):
                        nkt = 4 * qi + 4
                        for kj in range(nkt):
                            if kj == 4 * qi + 3:
                                continue  # merged into the packed step
                            steps.append((pair, qi, kj, nkt))

                # ctx deferral: while pair p's v fillers are pending, hold all
                # of pair p's ctx matmuls (exp output parked in pt/pt8 pools);
                # flush in window order once the reshapes are emitted.  This
                # keeps the in-order PE queue from blocking on the v pipeline.
                deferred = []
                defer_done = [False, False]

                cps_map = {}
                pt8_map = {}
                flushed = [False]

                def get_cps(pair, qi):
                    if (pair, qi) not in cps_map:
                        cps_map[(pair, qi)] = [
                            ctxps.tile([DK + 1, QW], f32, tag=f"ctx{h2}",
                                       name=f"ctx{h2}", bufs=1)
                            for h2 in range(2)
                        ]
                    return cps_map[(pair, qi)]

                def normalize_cols(pair, qi, c0, c1):
                    cps = cps_map[(pair, qi)]
                    w = c1 - c0
                    for h2 in range(2):
                        rec = small.tile([1, QW], f32, tag="rec")
                        nc.vector.reciprocal(rec[0:1, 0:w], cps[h2][64:65, c0:c1])
                        bc = small.tile([64, QW], f32, tag="bc")
                        nc.gpsimd.partition_broadcast(bc[:, 0:w], rec[0:1, 0:w], channels=64)
                        nc.vector.tensor_tensor(
                            out=ctxT[pair][
                                h2 * 64 : h2 * 64 + 64, qi * QW + c0 : qi * QW + c1
                            ],
                            in0=cps[h2][0:64, c0:c1],
                            in1=bc[:, 0:w],
                            op=MUL,
                        )

                def normalize(pair, qi, c0=0):
                    normalize_cols(pair, qi, c0, QW)
                    del cps_map[(pair, qi)]

                def emit_scores(idx):
                    if idx >= len(steps):
                        return None
                    if steps[idx][0] == 1 and not flushed[0]:
                        # pair-1 scores need qpair[1]/kpair[1]: drain the
                        # pair-0-critical filler list first
                        while fillers:
                            fillers.pop(0)()
                        flushed[0] = True
                    pair, qi, kj, nkt = steps[idx]
                    return scores(pair, qi, kj, packed=(kj == 4 * qi + 2))

                squeue = [emit_scores(0)]
                for si, (pair, qi, kj, nkt) in enumerate(steps):
                    s_cur = squeue.pop(0)
                    squeue.append(emit_scores(si + 1))
                    want_defer = not defer_done[pair]
                    d = kj - 4 * qi
                    off = max(0, 128 * d)
                    if d == 2:
                        # packed last-two diag steps: one exp + one mask TT
                        # covering kj (256 wide) and kj+1 (128 wide), both h2
                        pt = ptp.tile([128, 2 * QW], bf16, tag="ptduo")
                        s3 = (s_cur[:].rearrange("p (h x) -> p h x", h=2)
                              [:, :, 0:384])
                        p3 = (pt[:].rearrange("p (h x) -> p h x", h=2)
                              [:, :, 0:384])
                        nc.scalar.activation(p3, s3, Exp)
                        m4 = (pt[:].rearrange("p (h b x) -> p h b x", h=2, b=2)
                              [:, :, :, 0:128])
                        t4 = (tri01[:].rearrange("p (h x) -> p h x", h=2)
                              [:, :, None, :].broadcast_to([128, 2, 2, 128]))
                        nc.vector.tensor_tensor(out=m4, in0=m4, in1=t4, op=MUL)

                        def diag2_ctx(pair=pair, qi=qi, kj=kj, nkt=nkt, pt=pt):
                            cps = get_cps(pair, qi)
                            for b in range(2):
                                kjj = kj + b
                                off = 128 * (kjj - 4 * qi)
                                for h2 in range(2):
                                    h = 2 * pair + h2
                                    nc.tensor.matmul(
                                        cps[h2][:, off:],
                                        vh[h][:, kjj * 65 : kjj * 65 + 65],
                                        pt[:, h2 * QW + 256 * b : h2 * QW + 256 * b + QW - off],
                                        start=False,
                                        stop=(b == 1),
                                    )
                        if want_defer:
                            deferred.append(diag2_ctx)
                        else:
                            diag2_ctx()
                    elif d < 0:
                        # off-diagonal: exp into one half of a paired fp8
                        # tile; the pair fuses into ONE DoubleRow ctx matmul
                        # at the odd step (accuracy-safe off the diagonal)
                        e = kj & 1
                        if e == 0:
                            pt8_map[(pair, qi)] = ptp8.tile(
                                [128, 2048], f8, tag="pt8", name="pt8"
                            )
                        pt8 = pt8_map[(pair, qi)]
                        nc.scalar.activation(
                            pt8[:, e * 1024 : (e + 1) * 1024], s_cur[:], Exp
                        )
                        if e == 1:
                            def drow_ctx(pair=pair, qi=qi, kj=kj, pt8=pt8):
                                cps = get_cps(pair, qi)
                                r3 = pt8[:].rearrange("p (t h x) -> p h t x", t=2, h=2)
                                for h2 in range(2):
                                    h = 2 * pair + h2
                                    nc.tensor.matmul(
                                        cps[h2][:],
                                        vh8[h][:, (kj - 1) * 128 : (kj + 1) * 128]
                                        .rearrange("p (t m) -> p t m", t=2)[:, :, 0:65],
                                        r3[:, h2],
                                        start=(kj == 1),
                                        stop=False,
                                        perf_mode=DRow,
                                    )
                            if want_defer:
                                deferred.append(drow_ctx)
                            else:
                                drow_ctx()
                    else:
                        pt = ptp.tile([128, 2 * QW], bf16, tag="ptduo")
                        s3 = s_cur[:].rearrange("p (h x) -> p h x", h=2)[:, :, off:]
                        p3 = pt[:].rearrange("p (h x) -> p h x", h=2)[:, :, off:]
                        nc.scalar.activation(p3, s3, Exp)
                        # both heads' diagonal masks in one DVE instr
                        # (tri01 holds two side-by-side copies of the mask)
                        mv = pt[:].rearrange("p (h x) -> p h x", h=2)[
                            :, :, off : off + 128
                        ]
                        nc.vector.tensor_tensor(
                            out=mv,
                            in0=mv,
                            in1=tri01[:].rearrange("p (h x) -> p h x", h=2),
                            op=MUL,
                        )

                        def diag_ctx(pair=pair, qi=qi, kj=kj, nkt=nkt, off=off, pt=pt):
                            cps = get_cps(pair, qi)
                            for h2 in range(2):
                                h = 2 * pair + h2
                                nc.tensor.matmul(
                                    cps[h2][:, off:] if off else cps[h2][:],
                                    vh[h][:, kj * 65 : kj * 65 + 65],
                                    pt[:, h2 * QW + off : (h2 + 1) * QW],
                                    start=(kj == 0),
                                    stop=False,
                                )
                        if want_defer:
                            deferred.append(diag_ctx)
                        else:
                            diag_ctx()
                    if pair == 1 and qi == 3 and kj == nkt - 3:
                        # last window: cols [0,256) are final (kj 14/15 only
                        # write [256,512)): normalize the half and emit
                        # stiles 12/13 early to shorten the tail
                        normalize_cols(1, 3, 0, 256)
                        for st in (12, 13):
                            fillers2.append(lambda st=st: stile_og(st, 0, True))
                            fillers2.append(lambda st=st: stile_og(st, 1, True))
                    if kj == nkt - 2:
                        def win_end(pair=pair, qi=qi):
                            if pair == 1 and qi == 3:
                                normalize(pair, qi, c0=256)
                            else:
                                normalize(pair, qi)
                            if pair == 1 and qi < 3:
                                # out stiles for this window become fillers;
                                # emitted here (after normalize) so the read
                                # of ctxT is sequenced after its write
                                for st in range(qi * 4, qi * 4 + 4):
                                    fillers2.append(lambda st=st: stile_og(st, 0))
                                    fillers2.append(lambda st=st: stile_og(st, 1))
                        if want_defer:
                            deferred.append(win_end)
                        else:
                            win_end()
                    # pop fillers at step END: their DVE copies land after
                    # this step's mask/normalize in the in-order DVE queue,
                    # keeping the exp->mask->ctx critical path unobstructed
                    pops = 2 if vleft[pair] > 0 else (1 if si % 2 == 1 else 0)
                    if si >= 30 and vleft[1] > 0:
                        pops = max(pops, 2)
                    for _ in range(pops):
                        if fillers:
                            fillers.pop(0)()
                        elif (pair == 1 or si >= 30) and fillers2:
                            fillers2.pop(0)()
                    # flush deferred ctx (window order) once pair's v is in
                    if not defer_done[pair] and vleft[pair] == 0:
                        for th in deferred:
                            th()
                        deferred.clear()
                        defer_done[pair] = True
                while fillers:
                    fillers.pop(0)()
                for th in deferred:
                    th()
                deferred.clear()
                while fillers2:
                    fillers2.pop(0)()

            # tail: remaining out stiles in a fresh triple-buffered psum
            # scope (attention pools closed -> banks free), copies split
            # DVE/ACT so the stile chain pipelines
            with tc.tile_pool(name=f"tps{rep}", bufs=3, space="PSUM") as tps:
                for st in range(16):
                    if st not in emitted_st:
                        emit_out_stile(st, pool=tps, alt_copy=True)

    nc.finalize()
    return nc


_NC_CACHE = {}


def _get_program(repeat=1):
    phases = int(os.environ.get("KERNEL_PHASES", "3"))
    key = (repeat, phases)
    if key not in _NC_CACHE:
        _NC_CACHE[key] = _build_program(repeat, phases)
    return _NC_CACHE[key]


def _host_inputs(query, key, value, Wq, bq, Wk, bk, Wv, bv, Wo):
    """Build the 8 per-core input maps (numpy, host-side shard/transpose)."""
    import ml_dtypes

    bf16 = ml_dtypes.bfloat16
    query = np.asarray(query, dtype=np.float32)
    key = np.asarray(key, dtype=np.float32)
    value = np.asarray(value, dtype=np.float32)
    Wq = np.asarray(Wq, dtype=np.float32)
    Wk = np.asarray(Wk, dtype=np.float32)
    Wv = np.asarray(Wv, dtype=np.float32)
    Wo = np.asarray(Wo, dtype=np.float32)
    bq = np.asarray(bq, dtype=np.float32)
    bk = np.asarray(bk, dtype=np.float32)
    bv = np.asarray(bv, dtype=np.float32)

    scale = 1.0 / np.sqrt(np.float32(DK))

    def wtile(WT):  # [1024 i, 1024 f] -> [8, 2, 128, 512] (i-tile, f-half)
        return np.ascontiguousarray(
            WT.reshape(8, 128, 2, 512).transpose(0, 2, 1, 3)
        ).astype(bf16)

    wq4 = wtile(Wq.T * scale)
    wk4 = wtile(Wk.T)
    wv4 = wtile(Wv.T)
    WoT = np.ascontiguousarray(Wo.T)  # [i, o]

    bqr = (bq * scale).reshape(1, 1024).astype(bf16)
    bkr = bk.reshape(1, 1024).astype(bf16)
    bvr = bv.reshape(1, 1024).astype(bf16)
    t1 = np.triu(np.ones((128, 128), np.float32))
    tri01 = np.ascontiguousarray(np.concatenate([t1, t1], axis=1)).astype(bf16)

    in_maps = []
    for core in range(N_CORES):
        b, g = divmod(core, 4)
        sl = slice(g * ROWS_PER_CORE, (g + 1) * ROWS_PER_CORE)
        xq_ = np.ascontiguousarray(query[b, sl, :].T).astype(bf16).reshape(8, 128, QW)
        xk_ = np.ascontiguousarray(key[b, sl, :].T).astype(bf16).reshape(8, 128, QW)
        xv_ = np.ascontiguousarray(value[b, sl, :].T).astype(bf16).reshape(8, 128, QW)
        wo4 = np.ascontiguousarray(WoT[g * 256 : (g + 1) * 256, :]).astype(bf16).reshape(2, 128, 1024)
        in_maps.append(
            {
                "ones512": np.ones((1, 512), bf16),
                "xq": xq_,
                "xk": xk_,
                "xv": xv_,
                "wq": wq4,
                "wk": wk4,
                "wv": wv4,
                "wo": wo4,
                "bqr": bqr,
                "bkr": bkr,
                "bvr": bvr,
                "tri": tri01,
            }
        )
    return in_maps


def run_cores(in_maps, trace=False, trace_kwargs=None, repeat=1):
    """Compile + run the SPMD program on cores 0-7, return BassKernelResults."""
    from concourse.bass_utils import run_bass_kernel_spmd

    nc = _get_program(repeat)
    kwargs = {}
    if trace:
        kwargs["trace"] = True
        if trace_kwargs:
            kwargs["trace_kwargs"] = trace_kwargs
    return run_bass_kernel_spmd(nc, in_maps, core_ids=list(range(N_CORES)), **kwargs)


def kernel(query, key, value, mask, Wq, bq, Wk, bk, Wv, bv, Wo, bo, _trace=False):
    in_maps = _host_inputs(query, key, value, Wq, bq, Wk, bk, Wv, bv, Wo)
    res = run_cores(in_maps, trace=_trace)
    bo = np.asarray(bo, dtype=np.float32)
    out = np.zeros((B, S, D), dtype=np.float32)
    for core in range(N_CORES):
        b = core // 4
        out[b] += np.asarray(res.results[core]["out"], dtype=np.float32)
    out += bo[None, None, :]
    kernel.last_results = res
    return out
